# revision 1
# baseline (speedup 1.0000x reference)
"""4-layer GCN (ArithmeticCircuitGNN) on 8 Trainium2 NeuronCores.

Node-parallel, aggregation on the TENSOR engine: 12544-padded shard/core,
LN affine folded into weights on host.  Per GCN layer:
  AllGather(u) -> dma_gather(u[src]) in (chunk, dst-block) cells ->
  one-hot matmul S^T @ msg accumulated in PSUM per dst block, seeded with
  the self-loop term via an identity matmul -> evac (*dinv_dst, +bias,
  relu, +residual).  No DRAM scatter-add, no V planes.
Each AllGather is split in two shard-row halves: the first half fires
mid-way through the previous layer's block loop and hides under compute.
The one-hot S tiles (fp8, 1.0 entries) and the gather index table are
shared by all 4 layers (same graph).

kernel(**inputs) takes FULL numpy inputs, returns FULL [100000,128] out.
"""

import os
import numpy as np
import ml_dtypes

import concourse.bass as bass
import concourse.bacc as bacc
import concourse.mybir as mybir
import concourse.tile as tile
from concourse.bass_utils import run_bass_kernel_spmd

BF16 = ml_dtypes.bfloat16
FP8 = ml_dtypes.float8_e4m3

N = 100000
E = 300000
NCORES = 8
NLOC = 12500
NPAD = 12544          # 98 * 128
NT = 98
HALF = NPAD // 2      # 6272 = 49 * 128 (collective split point)
NBH = NT // 2         # 49 blocks per half
NG = NPAD * NCORES    # 100352
NGH = HALF * NCORES   # 50176 rows per uf half-tensor
CHUNK = HALF * 4      # 25088 rows (4 ranks' halves) per gather chunk
NCHUNK = 4            # A0 A1 B0 B1
D_IN, D_H, D_OUT = 128, 256, 128
EPS = 1e-5
CALLG = 8             # groups per gather call (8*128 = 1024 idxs, ring cap)
NQ = 4                # SWDGE queues
VT_AHEAD = 2          # self-term DMA prefetch distance

F32 = mybir.dt.float32
BF = mybir.dt.bfloat16
I16 = mybir.dt.int16
F8 = mybir.dt.float8e4

SKIP_AGG = bool(int(os.environ.get("KERNEL_SKIP_AGG", "0")))
SKIP_CC = bool(int(os.environ.get("KERNEL_SKIP_CC", "0")))


# ---------------------------------------------------------------- host prep

def _wrap16(idx):
    """[M] -> [128, M//16]: position i -> (i%16, i//16), replicated x8."""
    M = len(idx)
    w = np.zeros((128, M // 16), dtype=np.int16)
    t = idx.reshape(M // 16, 16).T
    for g in range(8):
        w[g * 16:(g + 1) * 16, :] = t
    return w


def _prep_graph(edge_index):
    """Cell (chunk, dst-block) schedule for one-hot-matmul aggregation.

    Chunks: A0/A1 = first shard-halves of ranks 0-3 / 4-7, B0/B1 = second
    halves, matching the split AllGather output tensors ufA/ufB.
    Returns (gidx[8], S[8], calls, block_groups, need_call, M, Gtot, dinv);
    calls = [(chunk, row_off, n_rows), ...] in emission order;
    block_groups[b] = [(call_idx, slot, G), ...];  schedule uniform across
    cores (group counts maxed over cores)."""
    src = np.asarray(edge_index[0], dtype=np.int64)
    dst = np.asarray(edge_index[1], dtype=np.int64)
    deg = np.bincount(dst, minlength=N).astype(np.float64) + 1.0
    dinv = (1.0 / np.sqrt(deg)).astype(np.float32)

    r_arr = dst // NLOC
    dloc = dst - r_arr * NLOC
    b_arr = dloc // 128
    drow = dloc - b_arr * 128
    srank = src // NLOC
    spad = src % NLOC                      # row within shard (pad ignored)
    half = (spad >= HALF).astype(np.int64)
    quad = srank // 4
    c_arr = half * 2 + quad
    crow = (srank % 4) * HALF + spad - half * HALF

    counts = np.bincount(
        (r_arr * NCHUNK + c_arr) * NT + b_arr,
        minlength=NCORES * NCHUNK * NT).reshape(NCORES, NCHUNK, NT)
    k = -(-counts.max(axis=0) // 128)        # [NCHUNK, NT] groups per cell
    ngroups = k.sum(axis=1)                  # per chunk
    base = np.zeros((NCHUNK, NT), np.int64)  # group base within chunk
    base[:, 1:] = np.cumsum(k, axis=1)[:, :-1]

    ncalls = [-(-int(g) // CALLG) for g in ngroups]
    merged = []                              # (chunk, local_call)
    for i in range(max(ncalls)):
        for c in range(NCHUNK):
            if i < ncalls[c]:
                merged.append((c, i))
    calls = []
    cidx = {}
    goff = {}                                # (c, local_call) -> global G base
    off = 0
    for ci, (c, i) in enumerate(merged):
        g0 = i * CALLG
        ng = min(CALLG, int(ngroups[c]) - g0)
        calls.append((c, off, ng * 128))
        cidx[(c, i)] = ci
        goff[(c, i)] = off // 128
        off += ng * 128
    M = off
    Gtot = M // 128

    # gmap[c, g_loc] -> global group id
    gmap = np.zeros((NCHUNK, max(1, int(ngroups.max()))), np.int64)
    callof = np.zeros_like(gmap)
    for c in range(NCHUNK):
        for g in range(int(ngroups[c])):
            i = g // CALLG
            gmap[c, g] = goff[(c, i)] + g % CALLG
            callof[c, g] = cidx[(c, i)]

    block_groups = [[] for _ in range(NT)]
    need_call = np.full(NT, -1, np.int64)
    for b in range(NT):
        for c in range(NCHUNK):
            for j in range(int(k[c, b])):
                g = int(base[c, b]) + j
                ci = int(callof[c, g])
                block_groups[b].append((ci, g % CALLG, int(gmap[c, g])))
                need_call[b] = max(need_call[b], ci)

    gidx, Ss = [], []
    for r in range(NCORES):
        m = r_arr == r
        ec, eb = c_arr[m], b_arr[m]
        ecrow, edrow = crow[m], drow[m]
        cell = ec * NT + eb
        order = np.argsort(cell, kind="stable")
        cell_s = cell[order]
        starts = np.searchsorted(cell_s, np.arange(NCHUNK * NT))
        tpos = np.arange(len(cell_s)) - starts[cell_s]
        g_loc = base.reshape(-1)[cell_s] + tpos // 128
        G = gmap[cell_s // NT, g_loc]
        row = G * 128 + tpos % 128
        g_rows = np.zeros(M, np.int16)
        g_rows[row] = ecrow[order]
        S3 = np.zeros((128, Gtot, 128), FP8)
        S3[tpos % 128, G, edrow[order]] = 1.0
        gidx.append(_wrap16(g_rows))
        Ss.append(np.ascontiguousarray(S3.reshape(128, Gtot * 128)))
    return gidx, Ss, calls, block_groups, need_call, M, Gtot, dinv


def _rep(v, p=128):
    return np.ascontiguousarray(
        np.broadcast_to(np.asarray(v, np.float32), (p, len(v))))


MP_BUFS = 3


def _check_liveness(calls, block_groups, need_call):
    """Every matmul must read a msg tile within the last MP_BUFS of its
    chunk's pool tag, else pool rotation clobbers it."""
    emitted = {c: [] for c in range(NCHUNK)}
    ci = 0
    for b in range(NT):
        while ci <= need_call[b]:
            emitted[calls[ci][0]].append(ci)
            ci += 1
        for (cj, slot, G) in block_groups[b]:
            c = calls[cj][0]
            assert cj in emitted[c][-MP_BUFS:], (
                f"block {b} reads call {cj} beyond pool depth "
                f"{emitted[c][-MP_BUFS - 2:]}")


# ---------------------------------------------------------------- builder

def _build(M, Gtot, calls, block_groups, need_call,
           use_brow1, use_brow2, use_fg, use_fb):
    _check_liveness(calls, block_groups, need_call)
    nc = bacc.Bacc(None, target_bir_lowering=False, num_swdge_queues=NQ)

    def param(name, shape, dt, out=False):
        return nc.declare_dram_parameter(name, shape, dt, isOutput=out)

    u0_own = param("u0_own", [NPAD, D_IN], BF)
    gidx = param("gidx", [128, M // 16], I16)
    S_p = param("S", [128, Gtot * 128], F8)
    dinv_p = param("dinv", [128, NT], F32)
    ident_p = param("ident", [128, 128], BF)
    w0_p = param("w0", [128, D_H], BF)
    w1_p = param("w1", [2, 128, D_H], BF)
    w2_p = param("w2", [2, 128, D_H], BF)
    w3_p = param("w3", [2, 128, D_OUT], BF)
    b0_p = param("b0r", [128, D_H], F32)
    b1_p = param("b1r", [128, D_H], F32)
    b2_p = param("b2r", [128, D_H], F32)
    b3_p = param("b3r", [128, D_OUT], F32)
    brow1_p = param("brow1r", [128, D_H], F32) if use_brow1 else None
    brow2_p = param("brow2r", [128, D_H], F32) if use_brow2 else None
    fg_p = param("fgr", [128, D_OUT], F32) if use_fg else None
    fb_p = param("fbr", [128, D_OUT], F32) if use_fb else None
    out_p = param("out", [NLOC, D_OUT], F32, out=True)

    ul0 = nc.dram_tensor("ul0", [NPAD, D_IN], BF)
    ul23 = nc.dram_tensor("ul23", [NPAD, D_H], BF)
    ul4 = nc.dram_tensor("ul4", [NPAD, D_OUT], BF)
    uf0 = [nc.dram_tensor(f"uf0{h}", [NGH, D_IN], BF, addr_space="Shared")
           for h in "AB"]
    uf2 = [nc.dram_tensor(f"uf2{h}", [NGH, D_H], BF, addr_space="Shared")
           for h in "AB"]
    uf3 = [nc.dram_tensor(f"uf3{h}", [NGH, D_H], BF, addr_space="Shared")
           for h in "AB"]
    uf4 = [nc.dram_tensor(f"uf4{h}", [NGH, D_OUT], BF, addr_space="Shared")
           for h in "AB"]

    AX = mybir.AxisListType.X
    AF = mybir.ActivationFunctionType
    OP = mybir.AluOpType

    with tile.TileContext(nc) as tc:
        with (
            tc.tile_pool(name="const", bufs=1) as cp,
            tc.tile_pool(name="hbuf", bufs=1) as hp,
            tc.tile_pool(name="work", bufs=4) as wp,
            tc.tile_pool(name="vtp", bufs=VT_AHEAD + 2) as vp,
            tc.tile_pool(name="small", bufs=6) as sp,
            tc.tile_pool(name="msg", bufs=MP_BUFS) as mp,
            tc.tile_pool(name="psT", bufs=2, space="PSUM") as pT,
            tc.tile_pool(name="psM", bufs=2, space="PSUM") as pM,
            tc.tile_pool(name="psA", bufs=4, space="PSUM") as pA,
        ):
            def cload(par, shape, dt):
                t = cp.tile(shape, dt, tag=par.name)
                nc.sync.dma_start(t[:], par[:])
                return t

            def allgather_half(ul, uf, h):
                if SKIP_CC:
                    return
                nc.gpsimd.collective_compute(
                    "AllGather", OP.bypass,
                    ins=[ul[h * HALF:(h + 1) * HALF, :].opt()],
                    outs=[uf[h][:].opt()],
                    replica_groups=[list(range(NCORES))],
                )

            # Layer-1 collectives first (input staged to ul0: collectives
            # can't read IO tensors); they overlap the constant loads below.
            nc.sync.dma_start(ul0[0:HALF, :], u0_own[0:HALF, :])
            allgather_half(ul0, uf0, 0)
            nc.sync.dma_start(ul0[HALF:NPAD, :], u0_own[HALF:NPAD, :])
            allgather_half(ul0, uf0, 1)

            gi = cload(gidx, [128, M // 16], I16)
            S_sb = cload(S_p, [128, Gtot * 128], F8)
            S_v = S_sb.rearrange("p (g d) -> p g d", d=128)
            dv = cload(dinv_p, [128, NT], F32)
            idn = cload(ident_p, [128, 128], BF)
            w0 = cload(w0_p, [128, D_H], BF)

            def wload(par, d):
                t = cp.tile([128, 2, d], BF, tag=par.name)
                nc.sync.dma_start(t[:], par.rearrange("k p d -> p k d"))
                return t

            w1 = wload(w1_p, D_H)
            w2 = wload(w2_p, D_H)
            w3 = wload(w3_p, D_OUT)
            b0 = cload(b0_p, [128, D_H], F32)
            b1 = cload(b1_p, [128, D_H], F32)
            b2 = cload(b2_p, [128, D_H], F32)
            b3 = cload(b3_p, [128, D_OUT], F32)
            brow1 = cload(brow1_p, [128, D_H], F32) if use_brow1 else None
            brow2 = cload(brow2_p, [128, D_H], F32) if use_brow2 else None
            fg = cload(fg_p, [128, D_OUT], F32) if use_fg else None
            fb = cload(fb_p, [128, D_OUT], F32) if use_fb else None

            h_sb = hp.tile([128, NT, D_H], BF)

            def r3(t, d):
                return t.rearrange("(n p) d -> p n d", p=128)

            def emit_gather(ci, uf, d):
                """One gather call -> flat msg tile; returns [128,G,d] view."""
                (c, off, n) = calls[ci]
                t = mp.tile([128, CALLG * D_H], BF, tag=f"m{c}")
                tv = t.rearrange("p (g d) -> p g d", d=d)
                src = uf[c // 2][(c % 2) * CHUNK:(c % 2 + 1) * CHUNK, :]
                nc.gpsimd.dma_gather(
                    tv[:, : n // 128, :], src,
                    gi[:, off // 16:(off + n) // 16], n, n, d,
                    queue_num=ci % NQ,
                )
                return tv

            def agg_block(b, tiles, vt, d):
                """Self-term seed + one-hot matmuls for block b -> psum."""
                ps = pA.tile([128, d], F32, tag="agg")
                groups = [] if SKIP_AGG else block_groups[b]
                nc.tensor.matmul(ps[:], idn[:], vt[:],
                                 start=True, stop=not groups)
                for j, (ci, slot, G) in enumerate(groups):
                    nc.tensor.matmul(ps[:], S_v[:, G, :], tiles[ci][:, slot, :],
                                     start=False, stop=(j == len(groups) - 1))
                return ps

            def transpose_mm(z_bf, w, d_out, kchunks):
                mm = pM.tile([128, d_out], F32, tag="mm")
                zt_ps = pT.tile([128, kchunks, 128], BF, tag="zt_ps")
                for kk in range(kchunks):
                    nc.tensor.transpose(
                        zt_ps[:, kk, :], z_bf[:, kk * 128:(kk + 1) * 128],
                        idn[:])
                zt = wp.tile([128, kchunks, 128], BF, tag="zt")
                nc.scalar.activation(zt[:], zt_ps[:], AF.Copy)
                for kk in range(kchunks):
                    nc.tensor.matmul(mm[:], zt[:, kk, :],
                                     w[:, kk, :] if kchunks > 1 else w[:],
                                     start=(kk == 0), stop=(kk == kchunks - 1))
                return mm

            def layer_loop(uf, d, ul_self, emit_block, mid_cc=None):
                """Gathers + per-block aggregation, pipelined; mid_cc fires
                after the first NBH blocks (their next-layer u rows stored)."""
                tiles = {}
                vts = {}
                ci = 0

                def vt_dma(b):
                    vt = vp.tile([128, d], BF, tag="vt")
                    nc.sync.dma_start(vt[:], r3(ul_self, d)[:, b, :])
                    vts[b] = vt

                for b in range(min(VT_AHEAD, NT)):
                    vt_dma(b)
                for b in range(NT):
                    while ci <= need_call[b]:
                        tiles[ci] = emit_gather(ci, uf, d)
                        ci += 1
                    if b + VT_AHEAD < NT:
                        vt_dma(b + VT_AHEAD)
                    ps = agg_block(b, tiles, vts.pop(b), d)
                    emit_block(b, ps)
                    if b == NBH - 1 and mid_cc is not None:
                        mid_cc()

            # ================= Layer 1 ====================================
            def l1_block(b, ps):
                tbf = wp.tile([128, D_IN], BF, tag="z1")
                nc.scalar.activation(tbf[:], ps[:], AF.Copy)
                mm = transpose_mm(tbf, w0, D_H, 1)
                t2 = wp.tile([128, D_H], F32, tag="u")
                nc.vector.scalar_tensor_tensor(
                    t2[:], mm[:], dv[:, b:b + 1], b0[:], OP.mult, OP.add)
                nc.scalar.activation(h_sb[:, b, :], t2[:], AF.Relu)
                ln_mm_store(b, w1, brow1, ul23)

            def ln_mm_store(b, w, brow, ul):
                ht = h_sb[:, b, :]
                sums = sp.tile([128, 1], F32, tag="sums")
                nc.vector.tensor_reduce(sums[:], ht, AX, OP.add)
                negmu = sp.tile([128, 1], F32, tag="negmu")
                nc.vector.tensor_scalar_mul(negmu[:], sums[:], -1.0 / D_H)
                sq = wp.tile([128, D_H], F32, tag="sq")
                ssq = sp.tile([128, 1], F32, tag="ssq")
                nc.scalar.activation(sq[:], ht, AF.Square, bias=negmu[:],
                                     accum_out=ssq[:])
                varp = sp.tile([128, 1], F32, tag="varp")
                nc.vector.tensor_scalar(varp[:], ssq[:], 1.0 / D_H, EPS,
                                        OP.mult, OP.add)
                sd = sp.tile([128, 1], F32, tag="sd")
                nc.scalar.sqrt(sd[:], varp[:])
                rstd = sp.tile([128, 1], F32, tag="rstd")
                nc.vector.reciprocal(rstd[:], sd[:])
                s = sp.tile([128, 1], F32, tag="s")
                nc.vector.tensor_tensor(s[:], rstd[:], dv[:, b:b + 1],
                                        OP.mult)
                negmu_s = sp.tile([128, 1], F32, tag="negmu_s")
                nc.vector.tensor_tensor(negmu_s[:], negmu[:], s[:], OP.mult)
                z = wp.tile([128, D_H], BF, tag="z")
                nc.scalar.activation(z[:], ht, AF.Identity, bias=negmu_s[:],
                                     scale=s[:])
                mm = transpose_mm(z, w, D_H, 2)
                u = wp.tile([128, D_H], BF, tag="uu")
                if brow is not None:
                    nc.vector.scalar_tensor_tensor(
                        u[:], brow[:], dv[:, b:b + 1], mm[:],
                        OP.mult, OP.add)
                else:
                    nc.scalar.activation(u[:], mm[:], AF.Copy)
                nc.sync.dma_start(r3(ul, D_H)[:, b, :], u[:])

            layer_loop(uf0, D_IN, u0_own, l1_block,
                       mid_cc=lambda: allgather_half(ul23, uf2, 0))
            allgather_half(ul23, uf2, 1)

            # ================= Layers 2, 3 ================================
            def mk_mid_block(bias, next_fn):
                def mid_block(b, ps):
                    t4 = wp.tile([128, D_H], F32, tag="t4")
                    nc.vector.scalar_tensor_tensor(
                        t4[:], ps[:], dv[:, b:b + 1], bias[:],
                        OP.mult, OP.add)
                    r = wp.tile([128, D_H], F32, tag="r")
                    nc.scalar.activation(r[:], t4[:], AF.Relu)
                    nc.vector.tensor_tensor(h_sb[:, b, :], r[:],
                                            h_sb[:, b, :], OP.add)
                    next_fn(b)
                return mid_block

            layer_loop(uf2, D_H, ul23,
                       mk_mid_block(b1, lambda b: ln_mm_store(b, w2, brow2,
                                                              ul23)),
                       mid_cc=lambda: allgather_half(ul23, uf3, 0))
            allgather_half(ul23, uf3, 1)

            def l3_next(b):
                # u-compute for layer 4: (h * dinv) @ W3  (no LN)
                z = wp.tile([128, D_H], BF, tag="z")
                nc.vector.tensor_scalar_mul(z[:], h_sb[:, b, :],
                                            dv[:, b:b + 1])
                mm = transpose_mm(z, w3, D_OUT, 2)
                u = wp.tile([128, D_OUT], BF, tag="uu")
                nc.scalar.activation(u[:], mm[:], AF.Copy)
                nc.sync.dma_start(r3(ul4, D_OUT)[:, b, :], u[:])

            layer_loop(uf3, D_H, ul23, mk_mid_block(b2, l3_next),
                       mid_cc=lambda: allgather_half(ul4, uf4, 0))
            allgather_half(ul4, uf4, 1)

            # ================= Layer 4 ====================================
            def l4_block(b, ps):
                y2 = wp.tile([128, D_OUT], F32, tag="t4")
                nc.vector.scalar_tensor_tensor(
                    y2[:], ps[:], dv[:, b:b + 1], b3[:], OP.mult, OP.add)
                sums = sp.tile([128, 1], F32, tag="sums")
                nc.vector.tensor_reduce(sums[:], y2[:], AX, OP.add)
                negmu = sp.tile([128, 1], F32, tag="negmu")
                nc.vector.tensor_scalar_mul(negmu[:], sums[:], -1.0 / D_OUT)
                sq = wp.tile([128, D_OUT], F32, tag="sq")
                ssq = sp.tile([128, 1], F32, tag="ssq")
                nc.scalar.activation(sq[:], y2[:], AF.Square, bias=negmu[:],
                                     accum_out=ssq[:])
                varp = sp.tile([128, 1], F32, tag="varp")
                nc.vector.tensor_scalar(varp[:], ssq[:], 1.0 / D_OUT, EPS,
                                        OP.mult, OP.add)
                sd = sp.tile([128, 1], F32, tag="sd")
                nc.scalar.sqrt(sd[:], varp[:])
                rstd = sp.tile([128, 1], F32, tag="rstd")
                nc.vector.reciprocal(rstd[:], sd[:])
                zo = wp.tile([128, D_OUT], F32, tag="r")
                nc.vector.tensor_scalar(zo[:], y2[:], negmu[:], rstd[:],
                                        OP.add, OP.mult)
                if fg is not None:
                    zo2 = wp.tile([128, D_OUT], F32, tag="zo2")
                    nc.vector.tensor_tensor(zo2[:], zo[:], fg[:], OP.mult)
                    zo = zo2
                if fb is not None:
                    zo3 = wp.tile([128, D_OUT], F32, tag="zo3")
                    nc.vector.tensor_tensor(zo3[:], zo[:], fb[:], OP.add)
                    zo = zo3
                lo = b * 128
                nrow = min(128, NLOC - lo)
                if nrow > 0:
                    nc.sync.dma_start(out_p[lo:lo + nrow, :], zo[0:nrow, :])

            layer_loop(uf4, D_OUT, ul4, l4_block)

    nc.compile()
    return nc


_CACHE = {}


def kernel(x, edge_index, W0, b0, W1, b1, W2, b2, W3, b3,
           ln0_g, ln0_b, ln1_g, ln1_b, fln_g, fln_b):
    x = np.asarray(x, np.float32)
    edge_index = np.asarray(edge_index)
    (gidx, Ss, calls, block_groups, need_call, M, Gtot,
     dinv) = _prep_graph(edge_index)

    W1f = np.asarray(ln0_g, np.float32)[:, None] * np.asarray(W1, np.float32)
    W2f = np.asarray(ln1_g, np.float32)[:, None] * np.asarray(W2, np.float32)
    brow1 = np.asarray(ln0_b, np.float32) @ np.asarray(W1, np.float32)
    brow2 = np.asarray(ln1_b, np.float32) @ np.asarray(W2, np.float32)
    use_brow1 = bool(np.any(brow1 != 0))
    use_brow2 = bool(np.any(brow2 != 0))
    use_fg = bool(np.any(np.asarray(fln_g) != 1))
    use_fb = bool(np.any(np.asarray(fln_b) != 0))

    key = (M, Gtot, tuple(calls), tuple(need_call),
           tuple(tuple(g) for g in block_groups),
           use_brow1, use_brow2, use_fg, use_fb)
    if key not in _CACHE:
        _CACHE[key] = _build(M, Gtot, calls, block_groups, need_call,
                             use_brow1, use_brow2, use_fg, use_fb)
    nc = _CACHE[key]

    u0 = dinv[:, None].astype(np.float32) * x
    u0p = np.zeros((NCORES, NPAD, D_IN), BF16)
    for r in range(NCORES):
        u0p[r, :NLOC] = u0[r * NLOC:(r + 1) * NLOC]
    dinv_pad = np.zeros((NCORES, NPAD), np.float32)
    for r in range(NCORES):
        dinv_pad[r, :NLOC] = dinv[r * NLOC:(r + 1) * NLOC]

    def chunk2(Wf):
        return np.stack([Wf[0:128], Wf[128:256]]).astype(BF16)

    common = {
        "ident": np.eye(128, dtype=BF16),
        "w0": np.asarray(W0, np.float32).astype(BF16),
        "w1": chunk2(W1f), "w2": chunk2(W2f),
        "w3": chunk2(np.asarray(W3, np.float32)),
        "b0r": _rep(b0), "b1r": _rep(b1), "b2r": _rep(b2), "b3r": _rep(b3),
    }
    if use_brow1:
        common["brow1r"] = _rep(brow1)
    if use_brow2:
        common["brow2r"] = _rep(brow2)
    if use_fg:
        common["fgr"] = _rep(fln_g)
    if use_fb:
        common["fbr"] = _rep(fln_b)

    in_maps = []
    for r in range(NCORES):
        m = dict(common)
        m["u0_own"] = u0p[r]
        m["gidx"] = gidx[r]
        m["S"] = Ss[r]
        m["dinv"] = np.ascontiguousarray(dinv_pad[r].reshape(NT, 128).T)
        in_maps.append(m)

    res = run_bass_kernel_spmd(nc, in_maps, core_ids=list(range(NCORES)))
    out = np.concatenate([res.results[r]["out"] for r in range(NCORES)],
                         axis=0)
    return out.astype(np.float32)



# revision 5
# speedup vs baseline: 1.0941x; 1.0941x over previous
"""4-layer GCN (ArithmeticCircuitGNN) on 8 Trainium2 NeuronCores.

Node-parallel, aggregation on the TENSOR engine: 12544-padded shard/core,
LN affine folded into weights on host.  Per GCN layer:
  AllGather(u) -> dma_gather(u[src]) in (chunk, dst-block) cells ->
  one-hot matmul S^T @ msg accumulated in PSUM per dst block, seeded with
  the self-loop term via an identity matmul -> evac (*dinv_dst, +bias,
  relu, +residual).  No DRAM scatter-add, no V planes.
Each AllGather is split in two shard-row halves: the first half fires
mid-way through the previous layer's block loop and hides under compute.
The one-hot S tiles (fp8, 1.0 entries) and the gather index table are
shared by all 4 layers (same graph).

v2: the two d_h=256 layers (uf2/uf3) carry fp8 messages on the wire and
through the gather + one-hot matmuls (self-loop seed stays bf16 via a
separate bf16 copy of u), halving collective bytes, gather HBM traffic
and PE one-hot time.  The per-block LN/activation chain is fused:
relu+residual+mean-accum in one scalar_tensor_tensor, Square+var-accum
in one activation, bias/scale LN apply in one activation.

kernel(**inputs) takes FULL numpy inputs, returns FULL [100000,128] out.
"""

import os
import numpy as np
import ml_dtypes

import concourse.bass as bass
import concourse.bacc as bacc
import concourse.mybir as mybir
import concourse.tile as tile
from concourse.bass_utils import run_bass_kernel_spmd

BF16 = ml_dtypes.bfloat16
FP8 = ml_dtypes.float8_e4m3

N = 100000
E = 300000
NCORES = 8
NLOC = 12500
NPAD = 12544          # 98 * 128
NT = 98
HALF = NPAD // 2      # 6272 = 49 * 128 (collective split point)
NBH = NT // 2         # 49 blocks per half
NG = NPAD * NCORES    # 100352
NGH = HALF * NCORES   # 50176 rows per uf half-tensor
CHUNK = HALF * 4      # 25088 rows (4 ranks' halves) per gather chunk
NCHUNK = 4            # A0 A1 B0 B1
D_IN, D_H, D_OUT = 128, 256, 128
EPS = 1e-5
CALLG = 8             # groups per gather call (8*128 = 1024 idxs, ring cap)
NQ = 4                # SWDGE queues
VT_AHEAD = 2          # self-term DMA prefetch distance

F32 = mybir.dt.float32
BF = mybir.dt.bfloat16
I16 = mybir.dt.int16
F8 = mybir.dt.float8e4

SKIP_AGG = bool(int(os.environ.get("KERNEL_SKIP_AGG", "0")))
SKIP_CC = bool(int(os.environ.get("KERNEL_SKIP_CC", "0")))


# ---------------------------------------------------------------- host prep

def _wrap16(idx):
    """[M] -> [128, M//16]: position i -> (i%16, i//16), replicated x8."""
    M = len(idx)
    w = np.zeros((128, M // 16), dtype=np.int16)
    t = idx.reshape(M // 16, 16).T
    for g in range(8):
        w[g * 16:(g + 1) * 16, :] = t
    return w


def _prep_graph(edge_index):
    """Cell (chunk, dst-block) schedule for one-hot-matmul aggregation.

    Chunks: A0/A1 = first shard-halves of ranks 0-3 / 4-7, B0/B1 = second
    halves, matching the split AllGather output tensors ufA/ufB.
    Returns (gidx[8], S[8], calls, block_groups, need_call, M, Gtot, dinv);
    calls = [(chunk, row_off, n_rows), ...] in emission order;
    block_groups[b] = [(call_idx, slot, G), ...];  schedule uniform across
    cores (group counts maxed over cores)."""
    src = np.asarray(edge_index[0], dtype=np.int64)
    dst = np.asarray(edge_index[1], dtype=np.int64)
    deg = np.bincount(dst, minlength=N).astype(np.float64) + 1.0
    dinv = (1.0 / np.sqrt(deg)).astype(np.float32)

    r_arr = dst // NLOC
    dloc = dst - r_arr * NLOC
    b_arr = dloc // 128
    drow = dloc - b_arr * 128
    srank = src // NLOC
    spad = src % NLOC                      # row within shard (pad ignored)
    half = (spad >= HALF).astype(np.int64)
    quad = srank // 4
    c_arr = half * 2 + quad
    crow = (srank % 4) * HALF + spad - half * HALF

    counts = np.bincount(
        (r_arr * NCHUNK + c_arr) * NT + b_arr,
        minlength=NCORES * NCHUNK * NT).reshape(NCORES, NCHUNK, NT)
    k = -(-counts.max(axis=0) // 128)        # [NCHUNK, NT] groups per cell
    ngroups = k.sum(axis=1)                  # per chunk
    base = np.zeros((NCHUNK, NT), np.int64)  # group base within chunk
    base[:, 1:] = np.cumsum(k, axis=1)[:, :-1]

    ncalls = [-(-int(g) // CALLG) for g in ngroups]
    merged = []                              # (chunk, local_call)
    for i in range(max(ncalls)):
        for c in range(NCHUNK):
            if i < ncalls[c]:
                merged.append((c, i))
    calls = []
    cidx = {}
    goff = {}                                # (c, local_call) -> global G base
    off = 0
    for ci, (c, i) in enumerate(merged):
        g0 = i * CALLG
        ng = min(CALLG, int(ngroups[c]) - g0)
        calls.append((c, off, ng * 128))
        cidx[(c, i)] = ci
        goff[(c, i)] = off // 128
        off += ng * 128
    M = off
    Gtot = M // 128

    # gmap[c, g_loc] -> global group id
    gmap = np.zeros((NCHUNK, max(1, int(ngroups.max()))), np.int64)
    callof = np.zeros_like(gmap)
    for c in range(NCHUNK):
        for g in range(int(ngroups[c])):
            i = g // CALLG
            gmap[c, g] = goff[(c, i)] + g % CALLG
            callof[c, g] = cidx[(c, i)]

    block_groups = [[] for _ in range(NT)]
    need_call = np.full(NT, -1, np.int64)
    for b in range(NT):
        for c in range(NCHUNK):
            for j in range(int(k[c, b])):
                g = int(base[c, b]) + j
                ci = int(callof[c, g])
                block_groups[b].append((ci, g % CALLG, int(gmap[c, g])))
                need_call[b] = max(need_call[b], ci)

    gidx, Ss = [], []
    for r in range(NCORES):
        m = r_arr == r
        ec, eb = c_arr[m], b_arr[m]
        ecrow, edrow = crow[m], drow[m]
        cell = ec * NT + eb
        order = np.argsort(cell, kind="stable")
        cell_s = cell[order]
        starts = np.searchsorted(cell_s, np.arange(NCHUNK * NT))
        tpos = np.arange(len(cell_s)) - starts[cell_s]
        g_loc = base.reshape(-1)[cell_s] + tpos // 128
        G = gmap[cell_s // NT, g_loc]
        row = G * 128 + tpos % 128
        g_rows = np.zeros(M, np.int16)
        g_rows[row] = ecrow[order]
        S3 = np.zeros((128, Gtot, 128), FP8)
        S3[tpos % 128, G, edrow[order]] = 1.0
        gidx.append(_wrap16(g_rows))
        Ss.append(np.ascontiguousarray(S3.reshape(128, Gtot * 128)))
    return gidx, Ss, calls, block_groups, need_call, M, Gtot, dinv


def _rep(v, p=128):
    return np.ascontiguousarray(
        np.broadcast_to(np.asarray(v, np.float32), (p, len(v))))


MP_BUFS = 3


def _check_liveness(calls, block_groups, need_call):
    """Every matmul must read a msg tile within the last MP_BUFS of its
    chunk's pool tag, else pool rotation clobbers it."""
    emitted = {c: [] for c in range(NCHUNK)}
    ci = 0
    for b in range(NT):
        while ci <= need_call[b]:
            emitted[calls[ci][0]].append(ci)
            ci += 1
        for (cj, slot, G) in block_groups[b]:
            c = calls[cj][0]
            assert cj in emitted[c][-MP_BUFS:], (
                f"block {b} reads call {cj} beyond pool depth "
                f"{emitted[c][-MP_BUFS - 2:]}")


# ---------------------------------------------------------------- builder

def _build(M, Gtot, calls, block_groups, need_call,
           use_brow1, use_brow2, use_fg, use_fb):
    _check_liveness(calls, block_groups, need_call)
    nc = bacc.Bacc(None, target_bir_lowering=False, num_swdge_queues=NQ)

    def param(name, shape, dt, out=False):
        return nc.declare_dram_parameter(name, shape, dt, isOutput=out)

    u0_own = param("u0_own", [NPAD, D_IN], BF)
    gidx = param("gidx", [128, M // 16], I16)
    S_p = param("S", [128, Gtot * 128], F8)
    dinv_p = param("dinv", [128, NT], F32)
    ident_p = param("ident", [128, 128], BF)
    w0_p = param("w0", [128, D_H], BF)
    w1_p = param("w1", [2, 128, D_H], BF)
    w2_p = param("w2", [2, 128, D_H], BF)
    w3_p = param("w3", [2, 128, D_OUT], BF)
    b0_p = param("b0r", [128, D_H], F32)
    b1_p = param("b1r", [128, D_H], F32)
    b2_p = param("b2r", [128, D_H], F32)
    b3_p = param("b3r", [128, D_OUT], F32)
    brow1_p = param("brow1r", [128, D_H], F32) if use_brow1 else None
    brow2_p = param("brow2r", [128, D_H], F32) if use_brow2 else None
    fg_p = param("fgr", [128, D_OUT], F32) if use_fg else None
    fb_p = param("fbr", [128, D_OUT], F32) if use_fb else None
    out_p = param("out", [NLOC, D_OUT], F32, out=True)

    ul0 = nc.dram_tensor("ul0", [NPAD, D_IN], BF)
    ul23_bf = nc.dram_tensor("ul23_bf", [NPAD, D_H], BF)   # self-term source
    ul23_f8 = nc.dram_tensor("ul23_f8", [NPAD, D_H], F8)   # wire source
    ul4 = nc.dram_tensor("ul4", [NPAD, D_OUT], BF)
    uf0 = [nc.dram_tensor(f"uf0{h}", [NGH, D_IN], BF, addr_space="Shared")
           for h in "AB"]
    uf2 = [nc.dram_tensor(f"uf2{h}", [NGH, D_H], F8, addr_space="Shared")
           for h in "AB"]
    uf3 = [nc.dram_tensor(f"uf3{h}", [NGH, D_H], F8, addr_space="Shared")
           for h in "AB"]
    uf4 = [nc.dram_tensor(f"uf4{h}", [NGH, D_OUT], BF, addr_space="Shared")
           for h in "AB"]

    AX = mybir.AxisListType.X
    AF = mybir.ActivationFunctionType
    OP = mybir.AluOpType

    with tile.TileContext(nc) as tc:
        with (
            tc.tile_pool(name="const", bufs=1) as cp,
            tc.tile_pool(name="hbuf", bufs=1) as hp,
            tc.tile_pool(name="work", bufs=4) as wp,
            tc.tile_pool(name="vtp", bufs=VT_AHEAD + 2) as vp,
            tc.tile_pool(name="small", bufs=6) as sp,
            tc.tile_pool(name="msg", bufs=MP_BUFS) as mp,
            tc.tile_pool(name="psT", bufs=2, space="PSUM") as pT,
            tc.tile_pool(name="psM", bufs=2, space="PSUM") as pM,
            tc.tile_pool(name="psA", bufs=4, space="PSUM") as pA,
        ):
            def cload(par, shape, dt):
                t = cp.tile(shape, dt, tag=par.name)
                nc.sync.dma_start(t[:], par[:])
                return t

            def allgather_half(ul, uf, h):
                if SKIP_CC:
                    return
                nc.gpsimd.collective_compute(
                    "AllGather", OP.bypass,
                    ins=[ul[h * HALF:(h + 1) * HALF, :].opt()],
                    outs=[uf[h][:].opt()],
                    replica_groups=[list(range(NCORES))],
                )

            # Layer-1 collectives first (input staged to ul0: collectives
            # can't read IO tensors); they overlap the constant loads below.
            nc.sync.dma_start(ul0[0:HALF, :], u0_own[0:HALF, :])
            allgather_half(ul0, uf0, 0)
            nc.sync.dma_start(ul0[HALF:NPAD, :], u0_own[HALF:NPAD, :])
            allgather_half(ul0, uf0, 1)

            gi = cload(gidx, [128, M // 16], I16)
            S_sb = cload(S_p, [128, Gtot * 128], F8)
            S_v = S_sb.rearrange("p (g d) -> p g d", d=128)
            dv = cload(dinv_p, [128, NT], F32)
            idn = cload(ident_p, [128, 128], BF)
            w0 = cload(w0_p, [128, D_H], BF)

            def wload(par, d):
                t = cp.tile([128, 2, d], BF, tag=par.name)
                nc.sync.dma_start(t[:], par.rearrange("k p d -> p k d"))
                return t

            w1 = wload(w1_p, D_H)
            w2 = wload(w2_p, D_H)
            w3 = wload(w3_p, D_OUT)
            b0 = cload(b0_p, [128, D_H], F32)
            b1 = cload(b1_p, [128, D_H], F32)
            b2 = cload(b2_p, [128, D_H], F32)
            b3 = cload(b3_p, [128, D_OUT], F32)
            brow1 = cload(brow1_p, [128, D_H], F32) if use_brow1 else None
            brow2 = cload(brow2_p, [128, D_H], F32) if use_brow2 else None
            fg = cload(fg_p, [128, D_OUT], F32) if use_fg else None
            fb = cload(fb_p, [128, D_OUT], F32) if use_fb else None

            h_sb = hp.tile([128, NT, D_H], BF)

            def r3(t, d):
                return t.rearrange("(n p) d -> p n d", p=128)

            def emit_gather(ci, uf, d, dt):
                """One gather call -> flat msg tile; returns [128,G,d] view."""
                (c, off, n) = calls[ci]
                t = mp.tile([128, CALLG * d], dt, tag=f"m{c}")
                tv = t.rearrange("p (g d) -> p g d", d=d)
                src = uf[c // 2][(c % 2) * CHUNK:(c % 2 + 1) * CHUNK, :]
                nc.gpsimd.dma_gather(
                    tv[:, : n // 128, :], src,
                    gi[:, off // 16:(off + n) // 16], n, n, d,
                    queue_num=ci % NQ,
                )
                return tv

            def agg_block(b, tiles, vt, d):
                """Self-term seed + one-hot matmuls for block b -> psum."""
                ps = pA.tile([128, d], F32, tag="agg")
                groups = [] if SKIP_AGG else block_groups[b]
                nc.tensor.matmul(ps[:], idn[:], vt[:],
                                 start=True, stop=not groups)
                for j, (ci, slot, G) in enumerate(groups):
                    nc.tensor.matmul(ps[:], S_v[:, G, :], tiles[ci][:, slot, :],
                                     start=False, stop=(j == len(groups) - 1))
                return ps

            def transpose_mm(z_bf, w, d_out, kchunks):
                mm = pM.tile([128, d_out], F32, tag="mm")
                zt_ps = pT.tile([128, kchunks, 128], BF, tag="zt_ps")
                for kk in range(kchunks):
                    nc.tensor.transpose(
                        zt_ps[:, kk, :], z_bf[:, kk * 128:(kk + 1) * 128],
                        idn[:])
                zt = wp.tile([128, kchunks, 128], BF, tag="zt")
                if kchunks > 1:
                    nc.scalar.activation(zt[:, 0, :], zt_ps[:, 0, :], AF.Copy)
                    nc.vector.tensor_scalar_mul(zt[:, 1, :], zt_ps[:, 1, :],
                                                1.0)
                else:
                    nc.scalar.activation(zt[:], zt_ps[:], AF.Copy)
                for kk in range(kchunks):
                    nc.tensor.matmul(mm[:], zt[:, kk, :],
                                     w[:, kk, :] if kchunks > 1 else w[:],
                                     start=(kk == 0), stop=(kk == kchunks - 1))
                return mm

            def layer_loop(uf, d, dt, ul_self, emit_block, mid_cc=None):
                """Gathers + per-block aggregation, pipelined; mid_cc fires
                after the first NBH blocks (their next-layer u rows stored)."""
                tiles = {}
                vts = {}
                ci = 0

                def vt_dma(b):
                    vt = vp.tile([128, d], BF, tag="vt")
                    nc.sync.dma_start(vt[:], r3(ul_self, d)[:, b, :])
                    vts[b] = vt

                for b in range(min(VT_AHEAD, NT)):
                    vt_dma(b)
                for b in range(NT):
                    while ci <= need_call[b]:
                        tiles[ci] = emit_gather(ci, uf, d, dt)
                        ci += 1
                    if b + VT_AHEAD < NT:
                        vt_dma(b + VT_AHEAD)
                    ps = agg_block(b, tiles, vts.pop(b), d)
                    emit_block(b, ps)
                    if b == NBH - 1 and mid_cc is not None:
                        mid_cc()

            # ================= Layer 1 ====================================
            def l1_block(b, ps):
                tbf = wp.tile([128, D_IN], BF, tag="z1")
                nc.scalar.activation(tbf[:], ps[:], AF.Copy)
                mm = transpose_mm(tbf, w0, D_H, 1)
                t2 = wp.tile([128, D_H], F32, tag="u")
                nc.vector.scalar_tensor_tensor(
                    t2[:], mm[:], dv[:, b:b + 1], b0[:], OP.mult, OP.add)
                sums = sp.tile([128, 1], F32, tag="sums")
                nc.scalar.activation(h_sb[:, b, :], t2[:], AF.Relu,
                                     accum_out=sums[:])
                ln_mm_store(b, w1, brow1, sums)

            def ln_mm_store(b, w, brow, sums):
                """LN(h_sb[b]) (folded gain) * dinv -> @W -> u stored bf16
                (self-term) + fp8 (wire)."""
                ht = h_sb[:, b, :]
                negmu = sp.tile([128, 1], F32, tag="negmu")
                nc.vector.tensor_scalar_mul(negmu[:], sums[:], -1.0 / D_H)
                sq = wp.tile([128, D_H], F32, tag="sq")
                ssq = sp.tile([128, 1], F32, tag="ssq")
                nc.scalar.activation(sq[:], ht, AF.Square, bias=negmu[:],
                                     accum_out=ssq[:])
                varp = sp.tile([128, 1], F32, tag="varp")
                nc.vector.tensor_scalar(varp[:], ssq[:], 1.0 / D_H, EPS,
                                        OP.mult, OP.add)
                sd = sp.tile([128, 1], F32, tag="sd")
                nc.scalar.sqrt(sd[:], varp[:])
                rstd = sp.tile([128, 1], F32, tag="rstd")
                nc.vector.reciprocal(rstd[:], sd[:])
                s = sp.tile([128, 1], F32, tag="s")
                nc.vector.tensor_tensor(s[:], rstd[:], dv[:, b:b + 1],
                                        OP.mult)
                negmu_s = sp.tile([128, 1], F32, tag="negmu_s")
                nc.vector.tensor_tensor(negmu_s[:], negmu[:], s[:], OP.mult)
                z = wp.tile([128, D_H], BF, tag="z")
                nc.scalar.activation(z[:], ht, AF.Identity, bias=negmu_s[:],
                                     scale=s[:])
                mm = transpose_mm(z, w, D_H, 2)
                u = wp.tile([128, D_H], BF, tag="uu")
                if brow is not None:
                    nc.vector.scalar_tensor_tensor(
                        u[:], brow[:], dv[:, b:b + 1], mm[:],
                        OP.mult, OP.add)
                else:
                    nc.scalar.activation(u[:], mm[:], AF.Copy)
                nc.sync.dma_start(r3(ul23_bf, D_H)[:, b, :], u[:])
                u8 = wp.tile([128, D_H], F8, tag="u8")
                nc.vector.tensor_scalar_mul(u8[:], u[:], 1.0)
                nc.sync.dma_start(r3(ul23_f8, D_H)[:, b, :], u8[:])

            layer_loop(uf0, D_IN, BF, u0_own, l1_block,
                       mid_cc=lambda: allgather_half(ul23_f8, uf2, 0))
            allgather_half(ul23_f8, uf2, 1)

            # ================= Layers 2, 3 ================================
            def mk_mid_block(bias, next_fn):
                def mid_block(b, ps):
                    t4 = wp.tile([128, D_H], F32, tag="t4")
                    nc.vector.scalar_tensor_tensor(
                        t4[:], ps[:], dv[:, b:b + 1], bias[:],
                        OP.mult, OP.add)
                    sums = sp.tile([128, 1], F32, tag="sums")
                    # h = relu(t4) + h_old, and accumulate row sums for LN
                    nc.vector.scalar_tensor_tensor(
                        h_sb[:, b, :], t4[:], 0.0, h_sb[:, b, :],
                        OP.max, OP.add, accum_out=sums[:])
                    next_fn(b, sums)
                return mid_block

            layer_loop(uf2, D_H, F8, ul23_bf,
                       mk_mid_block(b1, lambda b, sums: ln_mm_store(
                           b, w2, brow2, sums)),
                       mid_cc=lambda: allgather_half(ul23_f8, uf3, 0))
            allgather_half(ul23_f8, uf3, 1)

            def l3_next(b, sums):
                # u-compute for layer 4: (h * dinv) @ W3  (no LN)
                z = wp.tile([128, D_H], BF, tag="z")
                nc.vector.tensor_scalar_mul(z[:], h_sb[:, b, :],
                                            dv[:, b:b + 1])
                mm = transpose_mm(z, w3, D_OUT, 2)
                u = wp.tile([128, D_OUT], BF, tag="uu")
                nc.scalar.activation(u[:], mm[:], AF.Copy)
                nc.sync.dma_start(r3(ul4, D_OUT)[:, b, :], u[:])

            layer_loop(uf3, D_H, F8, ul23_bf,
                       mk_mid_block(b2, l3_next),
                       mid_cc=lambda: allgather_half(ul4, uf4, 0))
            allgather_half(ul4, uf4, 1)

            # ================= Layer 4 ====================================
            def l4_block(b, ps):
                y2 = wp.tile([128, D_OUT], F32, tag="t4")
                sums = sp.tile([128, 1], F32, tag="sums")
                nc.vector.scalar_tensor_tensor(
                    y2[:], ps[:], dv[:, b:b + 1], b3[:], OP.mult, OP.add,
                    accum_out=sums[:])
                negmu = sp.tile([128, 1], F32, tag="negmu")
                nc.vector.tensor_scalar_mul(negmu[:], sums[:], -1.0 / D_OUT)
                sq = wp.tile([128, D_OUT], F32, tag="sq")
                ssq = sp.tile([128, 1], F32, tag="ssq")
                nc.scalar.activation(sq[:], y2[:], AF.Square, bias=negmu[:],
                                     accum_out=ssq[:])
                varp = sp.tile([128, 1], F32, tag="varp")
                nc.vector.tensor_scalar(varp[:], ssq[:], 1.0 / D_OUT, EPS,
                                        OP.mult, OP.add)
                sd = sp.tile([128, 1], F32, tag="sd")
                nc.scalar.sqrt(sd[:], varp[:])
                rstd = sp.tile([128, 1], F32, tag="rstd")
                nc.vector.reciprocal(rstd[:], sd[:])
                zo = wp.tile([128, D_OUT], F32, tag="r")
                nc.vector.tensor_scalar(zo[:], y2[:], negmu[:], rstd[:],
                                        OP.add, OP.mult)
                if fg is not None:
                    zo2 = wp.tile([128, D_OUT], F32, tag="zo2")
                    nc.vector.tensor_tensor(zo2[:], zo[:], fg[:], OP.mult)
                    zo = zo2
                if fb is not None:
                    zo3 = wp.tile([128, D_OUT], F32, tag="zo3")
                    nc.vector.tensor_tensor(zo3[:], zo[:], fb[:], OP.add)
                    zo = zo3
                lo = b * 128
                nrow = min(128, NLOC - lo)
                if nrow > 0:
                    nc.sync.dma_start(out_p[lo:lo + nrow, :], zo[0:nrow, :])

            layer_loop(uf4, D_OUT, BF, ul4, l4_block)

    nc.compile()
    return nc


_CACHE = {}


def kernel(x, edge_index, W0, b0, W1, b1, W2, b2, W3, b3,
           ln0_g, ln0_b, ln1_g, ln1_b, fln_g, fln_b):
    x = np.asarray(x, np.float32)
    edge_index = np.asarray(edge_index)
    (gidx, Ss, calls, block_groups, need_call, M, Gtot,
     dinv) = _prep_graph(edge_index)

    W1f = np.asarray(ln0_g, np.float32)[:, None] * np.asarray(W1, np.float32)
    W2f = np.asarray(ln1_g, np.float32)[:, None] * np.asarray(W2, np.float32)
    brow1 = np.asarray(ln0_b, np.float32) @ np.asarray(W1, np.float32)
    brow2 = np.asarray(ln1_b, np.float32) @ np.asarray(W2, np.float32)
    use_brow1 = bool(np.any(brow1 != 0))
    use_brow2 = bool(np.any(brow2 != 0))
    use_fg = bool(np.any(np.asarray(fln_g) != 1))
    use_fb = bool(np.any(np.asarray(fln_b) != 0))

    key = (M, Gtot, tuple(calls), tuple(need_call),
           tuple(tuple(g) for g in block_groups),
           use_brow1, use_brow2, use_fg, use_fb)
    if key not in _CACHE:
        _CACHE[key] = _build(M, Gtot, calls, block_groups, need_call,
                             use_brow1, use_brow2, use_fg, use_fb)
    nc = _CACHE[key]

    u0 = dinv[:, None].astype(np.float32) * x
    u0p = np.zeros((NCORES, NPAD, D_IN), BF16)
    for r in range(NCORES):
        u0p[r, :NLOC] = u0[r * NLOC:(r + 1) * NLOC]
    dinv_pad = np.zeros((NCORES, NPAD), np.float32)
    for r in range(NCORES):
        dinv_pad[r, :NLOC] = dinv[r * NLOC:(r + 1) * NLOC]

    def chunk2(Wf):
        return np.stack([Wf[0:128], Wf[128:256]]).astype(BF16)

    common = {
        "ident": np.eye(128, dtype=BF16),
        "w0": np.asarray(W0, np.float32).astype(BF16),
        "w1": chunk2(W1f), "w2": chunk2(W2f),
        "w3": chunk2(np.asarray(W3, np.float32)),
        "b0r": _rep(b0), "b1r": _rep(b1), "b2r": _rep(b2), "b3r": _rep(b3),
    }
    if use_brow1:
        common["brow1r"] = _rep(brow1)
    if use_brow2:
        common["brow2r"] = _rep(brow2)
    if use_fg:
        common["fgr"] = _rep(fln_g)
    if use_fb:
        common["fbr"] = _rep(fln_b)

    in_maps = []
    for r in range(NCORES):
        m = dict(common)
        m["u0_own"] = u0p[r]
        m["gidx"] = gidx[r]
        m["S"] = Ss[r]
        m["dinv"] = np.ascontiguousarray(dinv_pad[r].reshape(NT, 128).T)
        in_maps.append(m)

    res = run_bass_kernel_spmd(nc, in_maps, core_ids=list(range(NCORES)))
    out = np.concatenate([res.results[r]["out"] for r in range(NCORES)],
                         axis=0)
    return out.astype(np.float32)
